# revision 28
# baseline (speedup 1.0000x reference)
"""Trainium2 Bass kernel for a dense transformer block (B=2, T=2048, C=1024, H=16).

Sequence-sharded with folded causal pairing: core i owns query blocks
{i, 15-i} of each batch (512 rows total). LN1 is precomputed on the host
(normalized h = LN(x) with gains folded into the projection weights), so the
device pipeline per batch is: K^T / V / Q^T projections in fp8 DoubleRow,
attention in bf16 (scores -> exp -> causal mask -> AV with a folded softmax
denominator row in V), the Wp projection in fp8 DoubleRow, residual + LN2
on-device, and a bf16 MLP.

fp8 weights are scaled x64 on the host to stay clear of e4m3 subnormals; the
descales ride for free: scores exp uses scale=1/4096 (k and q both x64), the
AV denominator column is 4.0 (so y comes out as 16*y_true, a good fp8 range),
and the Wp eviction uses scale=1/1024 (64 from Wp, 16 from y).

Schedule: kvq0 | kvq1 interleaved with attn0 heads | attn1 heads interleaved
with Wp(b0)+LN2+h2-transposes | Wp(b1) | MLP. Attention is software-pipelined
(scores of head h+1 issue before AV of head h); PSUM: mm(2) ps(3) py(2) tr(1).
"""

import sys

sys.path.insert(0, "/opt/trn_rl_repo")

import ml_dtypes
import numpy as np

import concourse.bacc as bacc
import concourse.tile as tile
from concourse import mybir
from concourse.bass_utils import run_bass_kernel_spmd
from concourse.masks import make_identity

F = mybir.dt.float32
BF = mybir.dt.bfloat16
F8 = mybir.dt.float8e4
AF = mybir.ActivationFunctionType
OP = mybir.AluOpType
DR = mybir.MatmulPerfMode.DoubleRow

B, T, C, H, HD = 2, 2048, 1024, 16, 64
BT = B * T
D4 = 4 * C
P = 128
NBLK = T // P            # 16 query blocks of 128 rows per batch
NCORES = 8
TT = 512                 # token-tile width for the KVQ pass
NTT = T // TT
NO = C // P              # 8 contraction chunks (bf16); 4 DR chunks of 256
WS = 64.0                # fp8 weight scale (k/q/p)
WSV = 32.0               # fp8 weight scale for V (v values must stay under 448)

_CACHE = {}


def _copy(nc, eng, out, in_):
    """Plain PSUM->SBUF eviction on a chosen engine."""
    if eng == "dve":
        nc.vector.tensor_copy(out=out, in_=in_)
    elif eng == "act":
        nc.scalar.activation(out=out, in_=in_, func=AF.Copy)
    else:
        nc.gpsimd.tensor_copy(out=out, in_=in_)


def _build_program():
    nc = bacc.Bacc("TRN2", target_bir_lowering=False)

    hT = nc.dram_tensor("hT", [P, 4, 2, BT], F8, kind="ExternalInput")
    hqT = nc.dram_tensor("hqT", [P, 4, 2, 4 * P], F8, kind="ExternalInput")
    wk = nc.dram_tensor("wk", [P, 4, 2, C], F8, kind="ExternalInput")
    wq = nc.dram_tensor("wq", [P, 4, 2, C], F8, kind="ExternalInput")
    wv = nc.dram_tensor("wv", [P, 4, 2, C], F8, kind="ExternalInput")
    wp = nc.dram_tensor("wp", [P, 4, 2, C], F8, kind="ExternalInput")
    w1 = nc.dram_tensor("w1", [C, D4], BF, kind="ExternalInput")
    w2 = nc.dram_tensor("w2", [D4, C], BF, kind="ExternalInput")
    xq = nc.dram_tensor("xq", [4 * P, C], F, kind="ExternalInput")
    mask = nc.dram_tensor("mask", [NBLK, P, 2 * P], BF, kind="ExternalInput")
    out = nc.dram_tensor("out", [4 * P, C], F, kind="ExternalOutput")

    with tile.TileContext(nc) as tc:
        with tc.tile_pool(name="wts", bufs=1) as wtp, \
             tc.tile_pool(name="maskp", bufs=1) as mp_, \
             tc.tile_pool(name="small", bufs=3) as smp, \
             tc.tile_pool(name="pt", bufs=19) as ptp, \
             tc.tile_pool(name="smt", bufs=13) as smtp, \
             tc.tile_pool(name="yst", bufs=3) as ystp, \
             tc.tile_pool(name="ypool", bufs=1) as ypl:
            mm_cm = tc.tile_pool(name="mm", bufs=3, space="PSUM", side="left")
            mmp = mm_cm.__enter__()
            psp_cm = tc.tile_pool(name="ps", bufs=3, space="PSUM", side="right")
            psp = psp_cm.__enter__()
            pyp_cm = tc.tile_pool(name="py", bufs=2, space="PSUM", side="right")
            pyp = pyp_cm.__enter__()

            wk_sb = wtp.tile([P, 4, 2, C], F8, tag="wk")
            wv_sb = wtp.tile([P, 4, 2, C], F8, tag="wv")
            wq_sb = wtp.tile([P, 4, 2, C], F8, tag="wq")
            mask_sb = mp_.tile([P, NBLK, 2 * P], BF)

            st = {"k": [None] * B, "v": [None] * B, "q": [None] * B,
                  "y": [None] * B}
            st["y"][0] = ypl.tile([P, 4, 2, 2 * P], F8, tag="ysb0", name="ysb0")
            st["y"][1] = ypl.tile([P, 4, 2, 2 * P], F8, tag="ysb1", name="ysb1")

            kv0_cm = tc.tile_pool(name="kv0", bufs=1, side="left")
            kv0 = kv0_cm.__enter__()
            st["k"][0] = kv0.tile([P, NO, T], BF, tag="ksb0", name="ksb0")
            st["v"][0] = kv0.tile([P, NBLK, H * 65], F8, tag="vsb0", name="vsb0")
            st["q"][0] = kv0.tile([P, NO, 2 * P], BF, tag="qsb0", name="qsb0")

            kv1_cm = tc.tile_pool(name="kv1", bufs=1, side="right")
            kv1 = kv1_cm.__enter__()
            st["k"][1] = kv1.tile([P, NO, T], BF, tag="ksb1", name="ksb1")
            st["v"][1] = kv1.tile([P, NBLK, H * 65], F8, tag="vsb1", name="vsb1")
            st["q"][1] = kv1.tile([P, NO, 2 * P], BF, tag="qsb1", name="qsb1")

            def load_weights():
                nc.sync.dma_start(out=wk_sb[:, 0:1], in_=wk[:, 0:1, :, :])
                nc.sync.dma_start(out=wk_sb[:, 1:4], in_=wk[:, 1:4, :, :])
                nc.sync.dma_start(out=wv_sb, in_=wv[:, :, :, :])

            xin0_cm = tc.tile_pool(name="xin0", bufs=2, side="left")
            xp0 = xin0_cm.__enter__()
            # ---- kvq0 (standalone; evicts spread evenly) ----
            _kvq_tiles(nc, 0, st, hT, wk_sb, wv_sb, mmp, xp0,
                       after_first_dma=load_weights,
                       k_eng=("dve", "act"), v_eng=("dve", "act"))
            nc.sync.dma_start(out=wq_sb, in_=wq[:, :, :, :])
            nc.sync.dma_start(out=mask_sb, in_=mask.rearrange("k p q -> p k q"))
            _q_proj(nc, 0, st, hqT, wq_sb, mmp, xp0)
            xin0_cm.__exit__(None, None, None)

            # ---- interleave 1: kvq1 steps with attn0 heads ----
            xin1_cm = tc.tile_pool(name="xin1", bufs=2, side="left")
            xp1 = xin1_cm.__enter__()
            kvq1_steps = _kvq_step_list(nc, 1, st, hT, hqT, wk_sb, wv_sb,
                                        wq_sb, mmp, xp1)
            A0 = _AttnState(nc, 0, st, mask_sb, smp, pyp, ptp, smtp, ystp, psp)
            A1 = _AttnState(nc, 1, st, mask_sb, smp, pyp, ptp, smtp, ystp,
                            psp, pool_kb2=(3,), pool_kb4=(1,))
            for i in range(H):
                kvq1_steps[i]()
                A0.head(i)
            A0.flush()
            for i in range(5):
                A1.head(i)
            xin1_cm.__exit__(None, None, None)
            kv0_cm.__exit__(None, None, None)

            postsh_cm = tc.tile_pool(name="postsh", bufs=1, side="left")
            postsh = postsh_cm.__enter__()
            eps_sb = postsh.tile([P, 4, C], F, tag="eps")
            h2t_sb = postsh.tile([P, NO, 4 * P], BF, tag="h2t")
            eps128 = postsh.tile([P, 1], F, tag="eps128")
            nc.vector.memset(eps128, 1e-5)
            wp_sb = postsh.tile([P, 4, 2, C], F8, tag="wp")
            nc.sync.dma_start(out=wp_sb, in_=wp[:, :, :, :])
            xq_sb = postsh.tile([P, 4, C], F, tag="xqh")
            nc.sync.dma_start(
                out=xq_sb,
                in_=xq.rearrange("(rt p) c -> p rt c", p=P))

            w1s_cm = tc.tile_pool(name="w1stream", bufs=2, side="left")
            w1sp = w1s_cm.__enter__()
            pre_w1 = {}
            for hg in range(2):
                w1c = w1sp.tile([P, NO, 512], BF, tag="w1c")
                nc.sync.dma_start(
                    out=w1c,
                    in_=w1[:, hg * 512:(hg + 1) * 512].rearrange(
                        "(o p) j -> p o j", p=P))
                pre_w1[(hg, 0)] = w1c

            # ---- interleave 2: attn1 heads with Wp(b0) + LN2 + transposes ----
            ln01 = _ln2_chunks(nc, (0, 1), eps_sb, h2t_sb, eps128, smp)
            tail0 = _wp_chunks(nc, 0, st, xq_sb, eps_sb, wp_sb, mmp, smp) + ln01
            for i in range(5, H):
                A1.head(i)
                if i - 5 < len(tail0):
                    tail0[i - 5]()
            A1.flush()
            for j in range(H - 5, len(tail0)):
                tail0[j]()
            kv1_cm.__exit__(None, None, None)

            pyp_cm.__exit__(None, None, None)
            psp_cm.__exit__(None, None, None)

            pacc_cm = tc.tile_pool(name="pacc", bufs=1, space="PSUM",
                                   side="right")
            paccp = pacc_cm.__enter__()
            ptr_cm = tc.tile_pool(name="ptrp", bufs=2, space="PSUM",
                                  side="right")
            ptrp = ptr_cm.__enter__()

            mlpsb_cm = tc.tile_pool(name="mlpsb", bufs=1, side="left")
            mlpp = mlpsb_cm.__enter__()
            aT_sb = mlpp.tile([P, D4 // P, 4 * P], BF, tag="aT")

            def tail1():
                for fn in _wp_chunks(nc, 1, st, xq_sb, eps_sb, wp_sb, mmp,
                                     smp):
                    fn()
                for fn in _ln2_chunks(nc, (2, 3), eps_sb, h2t_sb, eps128,
                                      smp):
                    fn()

            _mlp1_half(nc, h2t_sb, w1, aT_sb, ptrp, w1sp, pre_w1, 0,
                       between=tail1)
            mm_cm.__exit__(None, None, None)

            _mlp(nc, tc, eps_sb, h2t_sb, w1, w2, out, paccp, ptrp,
                 w1sp, pre_w1, mlpp, aT_sb, None)

            mlpsb_cm.__exit__(None, None, None)
            ptr_cm.__exit__(None, None, None)
            pacc_cm.__exit__(None, None, None)
            w1s_cm.__exit__(None, None, None)
            postsh_cm.__exit__(None, None, None)

    nc.compile()
    return nc


def _kvq_tiles(nc, b, st, hT, wk_sb, wv_sb, mmp, xp, after_first_dma=None,
               k_eng=None, v_eng=None):
    """K^T and V projections for batch b over all token tiles (fp8 DR)."""
    v_sb = st["v"][b]
    v_heads = v_sb.rearrange("p r (h w) -> p r h w", w=65)
    for tt in range(NTT):
        ts_ = tt * TT
        ht = xp.tile([P, 4, 2, TT], F8, tag="ht")
        if tt == 0 and after_first_dma is not None:
            # split loads: o=0 slices land first so the first K chain starts early
            nc.sync.dma_start(out=ht[:, 0:1],
                              in_=hT[:, 0:1, :, b * T + ts_:b * T + ts_ + TT])
            after_first_dma()
            nc.sync.dma_start(out=ht[:, 1:4],
                              in_=hT[:, 1:4, :, b * T + ts_:b * T + ts_ + TT])
        else:
            nc.sync.dma_start(out=ht,
                              in_=hT[:, :, :, b * T + ts_:b * T + ts_ + TT])
        if tt == 0:
            # softmax-denominator column: WSV/16 (so yev = 16*y_true)
            nc.gpsimd.memset(v_heads[:, :, :, 64:65], WSV / 16.0)
        for q4 in range(4):
            _kvq_quarter(nc, b, st, ht, wk_sb, wv_sb, mmp, tt, q4,
                         k_eng, v_eng)


def _kvq_quarter(nc, b, st, ht, wk_sb, wv_sb, mmp, tt, q4, k_eng, v_eng):
    """A quarter token tile's worth of K (2 jt) + V (1 t4) projections."""
    k_sb, v_sb = st["k"][b], st["v"][b]
    v_heads = v_sb.rearrange("p r (h w) -> p r h w", w=65)
    ts_ = tt * TT
    for jj in range(2):
        jt = q4 * 2 + jj
        pk = mmp.tile([P, TT], F, tag="mm", name=f"pk{b}_{tt}_{jt}")
        for o in range(4):
            nc.tensor.matmul(pk, wk_sb[:, o, :, jt * P:(jt + 1) * P],
                             ht[:, o], start=(o == 0), stop=(o == 3),
                             perf_mode=DR)
        _copy(nc, k_eng[jt % len(k_eng)], k_sb[:, jt, ts_:ts_ + TT], pk)
    t4 = q4
    ridx = tt * (TT // P) + t4
    for nh in range(2):
        pv = mmp.tile([P, 512], F, tag="mm", name=f"pv{b}_{ridx}_{nh}")
        for o in range(4):
            nc.tensor.matmul(pv, ht[:, o, :, t4 * P:(t4 + 1) * P],
                             wv_sb[:, o, :, nh * 512:(nh + 1) * 512],
                             start=(o == 0), stop=(o == 3),
                             perf_mode=DR)
        pvv = pv.rearrange("p (h d) -> p h d", d=HD)
        _copy(nc, v_eng[nh % len(v_eng)],
              v_heads[:, ridx, nh * 8:(nh + 1) * 8, 0:HD], pvv)


def _q_proj(nc, b, st, hqT, wq_sb, mmp, xp):
    qT_sb = st["q"][b]
    hq = xp.tile([P, 4, 2, 2 * P], F8, tag="hq")
    nc.sync.dma_start(out=hq, in_=hqT[:, :, :, b * 2 * P:(b + 1) * 2 * P])
    for jt in range(NO):
        pq = mmp.tile([P, 2 * P], F, tag="mm", name=f"pq{b}_{jt}")
        for o in range(4):
            nc.tensor.matmul(pq, wq_sb[:, o, :, jt * P:(jt + 1) * P],
                             hq[:, o], start=(o == 0), stop=(o == 3),
                             perf_mode=DR)
        _copy(nc, "dve", qT_sb[:, jt, :], pq)


def _kvq_step_list(nc, b, st, hT, hqT, wk_sb, wv_sb, wq_sb, mmp, xp):
    """kvq for batch b as 16 callables (one per half token tile), for
    interleaving with attention heads. Evictions balanced for the
    concurrent-attention engine budget."""
    k_eng = ("dve",)
    v_eng = ("dve",)
    hts = {}
    steps = []
    for tt in range(NTT):
        for q4 in range(4):
            def mk(tt=tt, q4=q4):
                def fn():
                    if q4 == 0:
                        ts_ = tt * TT
                        ht = xp.tile([P, 4, 2, TT], F8, tag="ht")
                        nc.sync.dma_start(
                            out=ht,
                            in_=hT[:, :, :, b * T + ts_:b * T + ts_ + TT])
                        if tt == 0:
                            v_heads = st["v"][b].rearrange(
                                "p r (h w) -> p r h w", w=65)
                            nc.gpsimd.memset(v_heads[:, :, :, 64:65], WSV / 16.0)
                        hts[tt] = ht
                    _kvq_quarter(nc, b, st, hts[tt], wk_sb, wv_sb, mmp, tt,
                                 q4, k_eng, v_eng)
                    if tt == NTT - 1 and q4 == 3:
                        _q_proj(nc, b, st, hqT, wq_sb, mmp, xp)
                return fn
            steps.append(mk())
    return steps


class _AttnState:
    """Software-pipelined attention for one batch: issue scores/exp/mask for
    head h, then AV for head h-1."""

    def __init__(self, nc, b, st, mask_sb, smp, pyp, ptp, smtp, ystp, psp,
                 pool_kb2=(2, 3), pool_kb4=(1,)):
        self.nc = nc
        self.b = b
        self.st = st
        self.mask_sb = mask_sb
        self.smp, self.pyp, self.ptp = smp, pyp, ptp
        self.smtp, self.ystp, self.psp = smtp, ystp, psp
        self.pool_kb2, self.pool_kb4 = pool_kb2, pool_kb4
        self.pending = []

    def head(self, h):
        pts = self._scores(h)
        self.pending.append((h, pts))
        if len(self.pending) > 2:
            self._av(*self.pending.pop(0))

    def flush(self):
        while self.pending:
            self._av(*self.pending.pop(0))

    def _scores(self, h):
        nc, b = self.nc, self.b
        k_sb, qT_sb = self.st["k"][b], self.st["q"][b]
        mask_sb = self.mask_sb
        po = (h % 2) * 64
        jt = h // 2
        pts = []
        for kb2 in range(4):
            ps_ = self.psp.tile([P, 512], F, tag="ps", name=f"ps{b}_{h}_{kb2}")
            for half in range(2):
                kb = kb2 * 2 + half
                nc.tensor.matmul(ps_[:, half * 256:(half + 1) * 256],
                                 k_sb[po:po + 64, jt, kb * P:(kb + 1) * P],
                                 qT_sb[po:po + 64, jt, :], start=True, stop=True)
            pe = self.smtp.tile([P, 512], BF, tag="pe")
            nc.scalar.activation(out=pe, in_=ps_, func=AF.Exp,
                                 scale=1.0 / (WS * WS))
            pT = self.ptp.tile([P, 512], BF, tag="pT")
            eng = nc.gpsimd if kb2 in self.pool_kb2 else nc.vector
            eng.tensor_tensor(
                out=pT, in0=pe,
                in1=mask_sb[:, kb2 * 2:kb2 * 2 + 2, :].rearrange("p a q -> p (a q)"),
                op=OP.mult)
            pts.append(pT)
        for kb4 in range(2):
            ps_ = self.psp.tile([P, 512], F, tag="ps", name=f"psB{b}_{h}_{kb4}")
            for j in range(4):
                kb = 8 + kb4 * 4 + j
                nc.tensor.matmul(ps_[:, j * P:(j + 1) * P],
                                 k_sb[po:po + 64, jt, kb * P:(kb + 1) * P],
                                 qT_sb[po:po + 64, jt, P:2 * P],
                                 start=True, stop=True)
            pe = self.smtp.tile([P, 512], BF, tag="pe")
            nc.scalar.activation(out=pe, in_=ps_, func=AF.Exp,
                                 scale=1.0 / (WS * WS))
            pT = self.ptp.tile([P, 512], BF, tag="pT")
            eng = nc.gpsimd if kb4 in self.pool_kb4 else nc.vector
            eng.tensor_tensor(
                out=pT.rearrange("p (a q) -> p a q", a=4),
                in0=pe.rearrange("p (a q) -> p a q", a=4),
                in1=mask_sb[:, 8 + kb4 * 4:8 + (kb4 + 1) * 4, P:2 * P],
                op=OP.mult)
            pts.append(pT)
        return pts

    def _av(self, h, pts):
        nc, b = self.nc, self.b
        v_sb, yT_sb = self.st["v"][b], self.st["y"][b]
        py = self.pyp.tile([65, 2 * P], F, tag="py")
        for kb2 in range(4):
            for half in range(2):
                kb = kb2 * 2 + half
                nc.tensor.matmul(py, v_sb[:, kb, h * 65:h * 65 + 65],
                                 pts[kb2][:, half * 256:(half + 1) * 256],
                                 start=(kb == 0), stop=False)
        for kb4 in range(2):
            for j in range(4):
                kb = 8 + kb4 * 4 + j
                nc.tensor.matmul(py[:, P:2 * P],
                                 v_sb[:, kb, h * 65:h * 65 + 65],
                                 pts[4 + kb4][:, j * P:(j + 1) * P],
                                 start=False, stop=(kb == NBLK - 1))
        rec = self.smp.tile([1, 2 * P], F, tag="rec")
        nc.vector.reciprocal(out=rec, in_=py[64:65, :])
        recb = self.smp.tile([64, 2 * P], F, tag="recb")
        nc.gpsimd.partition_broadcast(recb, rec)
        yev = self.ystp.tile([64, 2 * P], F8, tag="yev")
        nc.vector.tensor_tensor(out=yev, in0=py[0:64, :], in1=recb,
                                op=OP.mult)
        nc.sync.dma_start(
            out=yT_sb[(h % 2) * 64:(h % 2) * 64 + 64, h // 4, (h // 2) % 2, :],
            in_=yev)


def _wp_chunks(nc, b, st, xq_sb, eps_sb, wp_sb, mmp, smp):
    """Wp projection (fp8 DR) + residual for batch b, as 4 chunk callables."""
    yT_sb = st["y"][b]
    chunks = []
    for th in range(2):
        for nh in range(2):
            def mk(th=th, nh=nh):
                def fn():
                    rt = 2 * b + th
                    pr = mmp.tile([P, 512], F, tag="mm",
                                  name=f"pr{b}_{th}_{nh}")
                    for o in range(4):
                        nc.tensor.matmul(
                            pr, yT_sb[:, o, :, th * P:(th + 1) * P],
                            wp_sb[:, o, :, nh * 512:(nh + 1) * 512],
                            start=(o == 0), stop=(o == 3), perf_mode=DR)
                    nc.vector.scalar_tensor_tensor(
                        out=eps_sb[:, rt, nh * 512:(nh + 1) * 512],
                        in0=pr, scalar=1.0 / (WS * 16.0),
                        in1=xq_sb[:, rt, nh * 512:(nh + 1) * 512],
                        op0=OP.mult, op1=OP.add)
                return fn
            chunks.append(mk())
    return chunks


def _ln2_chunks(nc, rts, eps_sb, h2t_sb, eps128, h2p):
    """LN2 stats+normalize and h2 DMA-transpose for the given rt row-blocks,
    as callables."""
    chunks = []
    for rt in rts:
        def mk_ln(rt=rt):
            def fn():
                stats = h2p.tile([P, 2, 6], F, tag="st2")
                nc.vector.bn_stats(out=stats[:, 0, :], in_=eps_sb[:, rt, 0:512])
                nc.vector.bn_stats(out=stats[:, 1, :],
                                   in_=eps_sb[:, rt, 512:1024])
                mv = h2p.tile([P, 2], F, tag="mv2")
                nc.vector.bn_aggr(out=mv, in_=stats)
                sd = h2p.tile([P, 1], F, tag="sd2")
                nc.scalar.activation(out=sd, in_=mv[:, 1:2], func=AF.Sqrt,
                                     bias=eps128)
                rstd2 = h2p.tile([P, 1], F, tag="rstd2")
                nc.vector.reciprocal(out=rstd2, in_=sd)
                h2 = h2p.tile([P, C], BF, tag=f"h2_{rt % 2}")
                nc.vector.tensor_scalar(out=h2, in0=eps_sb[:, rt, :],
                                        scalar1=mv[:, 0:1], scalar2=rstd2,
                                        op0=OP.subtract, op1=OP.mult)
                nc.sync.dma_start_transpose(
                    out=h2t_sb[:, :, rt * P:(rt + 1) * P], in_=h2)
            return fn
        chunks.append(mk_ln())
    return chunks


def _mlp1_half(nc, h2t_sb, w1, aT_sb, ptrp, wsp, pre_w1, half, between=None):
    """MLP1 for one 256-token half: aT[:, :, half] = gelu(W1^T @ h2T-half)."""
    cs = half * 2 * P
    for hg in range(D4 // 512):
        if (hg, half) in pre_w1:
            w1c = pre_w1[(hg, half)]
        else:
            w1c = wsp.tile([P, NO, 512], BF, tag="w1c")
            nc.sync.dma_start(
                out=w1c,
                in_=w1[:, hg * 512:(hg + 1) * 512].rearrange(
                    "(o p) j -> p o j", p=P))
            pre_w1[(hg, half)] = None
        for hi in range(4):
            ht = hg * 4 + hi
            pa = ptrp.tile([P, 2 * P], F, tag="pa", name=f"pa{ht}_{half}")
            for o in range(NO):
                nc.tensor.matmul(pa, w1c[:, o, hi * P:(hi + 1) * P],
                                 h2t_sb[:, o, cs:cs + 2 * P],
                                 start=(o == 0), stop=(o == NO - 1))
            nc.scalar.activation(out=aT_sb[:, ht, cs:cs + 2 * P], in_=pa,
                                 func=AF.Gelu)
        if between is not None and hg == 0:
            between()
            between = None


def _mlp(nc, tc, eps_sb, h2t_sb, w1, w2, out, paccp, ptrp, wsp, pre_w1,
         mlpp, aT_sb, between):
    """MLP over all 512 own rows + final residual + output DMA (bf16)."""
    if True:
        # ---- MLP1: aT = gelu(W1^T @ h2T), second half ----
        _mlp1_half(nc, h2t_sb, w1, aT_sb, ptrp, wsp, pre_w1, 1,
                   between=between)

        # ---- MLP2 + residual ----
        macc_cm = tc.tile_pool(name="macc", bufs=1, space="PSUM", side="left")
        maccp = macc_cm.__enter__()
        with tc.tile_pool(name="w2stream", bufs=2, side="left") as wsp2:
            out_sb = mlpp.tile([P, 4, C], F, tag="outsb")
            for nh in range(2):
                pms = [paccp.tile([P, 512], F, tag=f"acc{rt}",
                                  name=f"pm{nh}_{rt}")
                       for rt in range(2)]
                pms += [maccp.tile([P, 512], F, tag=f"acc{rt}",
                                   name=f"pm{nh}_{rt}")
                        for rt in range(2, 4)]
                for hg in range(D4 // 512):
                    w2c = wsp2.tile([P, 4, 512], BF, tag="w2c",
                                    name=f"w2c{nh}_{hg}")
                    nc.sync.dma_start(
                        out=w2c,
                        in_=w2[hg * 512:(hg + 1) * 512, nh * 512:(nh + 1) * 512]
                        .rearrange("(g p) j -> p g j", p=P))
                    for gi in range(4):
                        hc = hg * 4 + gi
                        for rt in range(4):
                            nc.tensor.matmul(
                                pms[rt], aT_sb[:, hc, rt * P:(rt + 1) * P],
                                w2c[:, gi, :],
                                start=(hc == 0), stop=(hc == D4 // P - 1))
                for rt in range(4):
                    nc.vector.tensor_tensor(
                        out=out_sb[:, rt, nh * 512:(nh + 1) * 512],
                        in0=pms[rt],
                        in1=eps_sb[:, rt, nh * 512:(nh + 1) * 512],
                        op=OP.add)
                nc.sync.dma_start(
                    out=out.rearrange("(rt p) c -> p rt c", p=P)[
                        :, :, nh * 512:(nh + 1) * 512],
                    in_=out_sb[:, :, nh * 512:(nh + 1) * 512])
        macc_cm.__exit__(None, None, None)


def _dr_pack(m):
    """[C_in, N] -> [128, 4, 2, N], channel c -> (c//256, (c%256)//128, c%128)."""
    cin, n = m.shape
    assert cin == C
    return np.ascontiguousarray(m.reshape(4, 2, P, n).transpose(2, 0, 1, 3))


def _host_prep(inputs):
    """Host-side LN1, fp8 DR packing, per-core in_maps."""
    ii = {k: np.asarray(v, dtype=np.float32) for k, v in inputs.items()}
    x = ii["x"]
    for bias in ("bq", "bk", "bv", "bp", "b1", "b2", "ln1_b", "ln2_b"):
        assert np.allclose(ii[bias], 0.0), f"nonzero {bias} unsupported"

    e4 = ml_dtypes.float8_e4m3fn
    xflat = x.reshape(BT, C)
    mu = xflat.mean(axis=1, keepdims=True)
    var = ((xflat - mu) ** 2).mean(axis=1, keepdims=True)
    h = (xflat - mu) / np.sqrt(var + 1e-5)

    g1 = ii["ln1_g"][:, None]
    wq_f = (g1 * ii["Wq"] / np.sqrt(HD)).astype(np.float32)
    wk_f = (g1 * ii["Wk"]).astype(np.float32)
    wv_f = (g1 * ii["Wv"]).astype(np.float32)
    g2 = ii["ln2_g"][:, None]
    w1_f = (g2 * ii["W1"]).astype(np.float32)

    hT = np.ascontiguousarray(h.T)  # [C, BT]

    shared = {
        "hT": _dr_pack(hT).astype(e4),
        "wk": _dr_pack(wk_f * WS).astype(e4),
        "wq": _dr_pack(wq_f * WS).astype(e4),
        "wv": _dr_pack(wv_f * WSV).astype(e4),
        "wp": _dr_pack(ii["Wp"] * WS).astype(e4),
        "w1": np.ascontiguousarray(w1_f.astype(ml_dtypes.bfloat16)),
        "w2": np.ascontiguousarray(ii["W2"].astype(ml_dtypes.bfloat16)),
    }

    in_maps = []
    core_rows = []
    kk = np.arange(P)[:, None]
    jj = np.arange(2 * P)[None, :]
    for core in range(NCORES):
        qbA, qbB = core, NBLK - 1 - core
        rows = np.concatenate([
            b * T + qb * P + np.arange(P)
            for b in range(B) for qb in (qbA, qbB)])
        core_rows.append(rows)
        xq_i = np.ascontiguousarray(xflat[rows])
        hq_i = np.ascontiguousarray(h[rows].T)  # [C, 512]
        qpos = np.where(jj < P, qbA * P + jj, qbB * P + (jj - P))
        m = np.empty((NBLK, P, 2 * P), np.float32)
        for kb in range(NBLK):
            kpos = kb * P + kk
            m[kb] = np.where(kpos <= qpos, 1.0, 0.0)
        in_maps.append(dict(
            shared, xq=xq_i,
            hqT=_dr_pack(hq_i).astype(e4),
            mask=np.ascontiguousarray(m.astype(ml_dtypes.bfloat16))))
    return in_maps, core_rows


def kernel(**inputs):
    if "nc" not in _CACHE:
        _CACHE["nc"] = _build_program()
    nc = _CACHE["nc"]
    in_maps, core_rows = _host_prep(inputs)
    res = run_bass_kernel_spmd(nc, in_maps, core_ids=list(range(NCORES)))
    out = np.empty((BT, C), np.float32)
    for core in range(NCORES):
        out[core_rows[core]] = res.results[core]["out"]
    return out.reshape(B, T, C)


if __name__ == "__main__":
    print("module loads OK")


# revision 35
# speedup vs baseline: 1.0250x; 1.0250x over previous
"""Trainium2 Bass kernel for a dense transformer block (B=2, T=2048, C=1024, H=16).

Sequence-sharded with folded causal pairing: core i owns query blocks
{i, 15-i} of each batch (512 rows total). LN1 is precomputed on the host
(normalized h = LN(x) with gains folded into the projection weights), so the
device pipeline per batch is: K^T / V / Q^T projections in fp8 DoubleRow,
attention in bf16 (scores -> exp -> causal mask -> AV with a folded softmax
denominator row in V), the Wp projection in fp8 DoubleRow, residual + LN2
on-device, and a bf16 MLP.

fp8 weights are scaled x64 on the host to stay clear of e4m3 subnormals; the
descales ride for free: scores exp uses scale=1/4096 (k and q both x64), the
AV denominator column is 4.0 (so y comes out as 16*y_true, a good fp8 range),
and the Wp eviction uses scale=1/1024 (64 from Wp, 16 from y).

Schedule: kvq0 | kvq1 interleaved with attn0 heads | attn1 heads interleaved
with Wp(b0)+LN2+h2-transposes | Wp(b1) | MLP. Attention is software-pipelined
(scores of head h+1 issue before AV of head h); PSUM: mm(2) ps(3) py(2) tr(1).
"""

import sys

sys.path.insert(0, "/opt/trn_rl_repo")

import ml_dtypes
import numpy as np

import concourse.bacc as bacc
import concourse.tile as tile
from concourse import mybir
from concourse.bass_utils import run_bass_kernel_spmd
from concourse.masks import make_identity

F = mybir.dt.float32
BF = mybir.dt.bfloat16
F8 = mybir.dt.float8e4
AF = mybir.ActivationFunctionType
OP = mybir.AluOpType
DR = mybir.MatmulPerfMode.DoubleRow

B, T, C, H, HD = 2, 2048, 1024, 16, 64
BT = B * T
D4 = 4 * C
P = 128
NBLK = T // P            # 16 query blocks of 128 rows per batch
NCORES = 8
TT = 512                 # token-tile width for the KVQ pass
NTT = T // TT
NO = C // P              # 8 contraction chunks (bf16); 4 DR chunks of 256
WS = 64.0                # fp8 weight scale (k/q/p)
WSV = 32.0               # fp8 weight scale for V (v values must stay under 448)

_CACHE = {}


def _copy(nc, eng, out, in_):
    """Plain PSUM->SBUF eviction on a chosen engine."""
    if eng == "dve":
        nc.vector.tensor_copy(out=out, in_=in_)
    elif eng == "act":
        nc.scalar.activation(out=out, in_=in_, func=AF.Copy)
    else:
        nc.gpsimd.tensor_copy(out=out, in_=in_)


def _build_program():
    nc = bacc.Bacc("TRN2", target_bir_lowering=False)

    hT = nc.dram_tensor("hT", [P, 4, 2, BT], F8, kind="ExternalInput")
    hqT = nc.dram_tensor("hqT", [P, 4, 2, 4 * P], F8, kind="ExternalInput")
    wk = nc.dram_tensor("wk", [P, 4, 2, C], F8, kind="ExternalInput")
    wq = nc.dram_tensor("wq", [P, 4, 2, C], F8, kind="ExternalInput")
    wv = nc.dram_tensor("wv", [P, 4, 2, C], F8, kind="ExternalInput")
    wp = nc.dram_tensor("wp", [P, 4, 2, C], F8, kind="ExternalInput")
    w1 = nc.dram_tensor("w1", [C, D4], BF, kind="ExternalInput")
    w2 = nc.dram_tensor("w2", [D4, C], BF, kind="ExternalInput")
    xq = nc.dram_tensor("xq", [4 * P, C], F, kind="ExternalInput")
    mask = nc.dram_tensor("mask", [NBLK, P, 2 * P], BF, kind="ExternalInput")
    out = nc.dram_tensor("out", [4 * P, C], F, kind="ExternalOutput")

    with tile.TileContext(nc) as tc:
        with tc.tile_pool(name="maskp", bufs=1) as mp_, \
             tc.tile_pool(name="small", bufs=3) as smp, \
             tc.tile_pool(name="pt", bufs=19) as ptp, \
             tc.tile_pool(name="smt", bufs=13) as smtp, \
             tc.tile_pool(name="yst", bufs=3) as ystp, \
             tc.tile_pool(name="ypool", bufs=1) as ypl:
            mm_cm = tc.tile_pool(name="mm", bufs=3, space="PSUM", side="left")
            mmp = mm_cm.__enter__()
            psp_cm = tc.tile_pool(name="ps", bufs=3, space="PSUM", side="right")
            psp = psp_cm.__enter__()
            pyp_cm = tc.tile_pool(name="py", bufs=2, space="PSUM", side="right")
            pyp = pyp_cm.__enter__()

            wts_cm = tc.tile_pool(name="wts", bufs=1, side="left")
            wtp = wts_cm.__enter__()
            wk_sb = wtp.tile([P, 4, 2, C], F8, tag="wk")
            wv_sb = wtp.tile([P, 4, 2, C], F8, tag="wv")
            wq_sb = wtp.tile([P, 4, 2, C], F8, tag="wq")
            mask_sb = mp_.tile([P, NBLK, 2 * P], BF)

            st = {"k": [None] * B, "v": [None] * B, "q": [None] * B,
                  "y": [None] * B}
            st["y"][0] = ypl.tile([P, 4, 2, 2 * P], F8, tag="ysb0", name="ysb0")
            st["y"][1] = ypl.tile([P, 4, 2, 2 * P], F8, tag="ysb1", name="ysb1")

            kv0_cm = tc.tile_pool(name="kv0", bufs=1, side="left")
            kv0 = kv0_cm.__enter__()
            st["k"][0] = kv0.tile([P, NO, T], BF, tag="ksb0", name="ksb0")
            st["v"][0] = kv0.tile([P, NBLK, H * 65], F8, tag="vsb0", name="vsb0")
            st["q"][0] = kv0.tile([P, NO, 2 * P], BF, tag="qsb0", name="qsb0")

            kv1_cm = tc.tile_pool(name="kv1", bufs=1, side="right")
            kv1 = kv1_cm.__enter__()
            st["k"][1] = kv1.tile([P, NO, T], BF, tag="ksb1", name="ksb1")
            st["v"][1] = kv1.tile([P, NBLK, H * 65], F8, tag="vsb1", name="vsb1")
            st["q"][1] = kv1.tile([P, NO, 2 * P], BF, tag="qsb1", name="qsb1")

            def load_weights():
                nc.sync.dma_start(out=wk_sb[:, 0:1], in_=wk[:, 0:1, :, :])
                nc.sync.dma_start(out=wk_sb[:, 1:4], in_=wk[:, 1:4, :, :])
                nc.sync.dma_start(out=wv_sb, in_=wv[:, :, :, :])

            xin0_cm = tc.tile_pool(name="xin0", bufs=2, side="left")
            xp0 = xin0_cm.__enter__()
            # ---- kvq0 (standalone; evicts spread evenly) ----
            _kvq_tiles(nc, 0, st, hT, wk_sb, wv_sb, mmp, xp0,
                       after_first_dma=load_weights,
                       k_eng=("dve", "act"), v_eng=("dve", "act"))
            nc.sync.dma_start(out=wq_sb, in_=wq[:, :, :, :])
            nc.sync.dma_start(out=mask_sb, in_=mask.rearrange("k p q -> p k q"))
            _q_proj(nc, 0, st, hqT, wq_sb, mmp, xp0)
            xin0_cm.__exit__(None, None, None)

            # ---- interleave 1: kvq1 steps with attn0 heads ----
            xin1_cm = tc.tile_pool(name="xin1", bufs=2, side="left")
            xp1 = xin1_cm.__enter__()
            kvq1_steps = _kvq_step_list(nc, 1, st, hT, hqT, wk_sb, wv_sb,
                                        wq_sb, mmp, xp1)
            A0 = _AttnState(nc, 0, st, mask_sb, smp, pyp, ptp, smtp, ystp, psp)
            A1 = _AttnState(nc, 1, st, mask_sb, smp, pyp, ptp, smtp, ystp,
                            psp, pool_kb2=(3,), pool_kb4=(1,))
            for i in range(H):
                kvq1_steps[i]()
                A0.head(i)
            A0.flush()
            for i in range(5):
                A1.head(i)
            xin1_cm.__exit__(None, None, None)
            kv0_cm.__exit__(None, None, None)
            wts_cm.__exit__(None, None, None)

            postsh_cm = tc.tile_pool(name="postsh", bufs=1, side="left")
            postsh = postsh_cm.__enter__()
            eps_sb = postsh.tile([P, 4, C], F, tag="eps")
            h2t_sb = postsh.tile([P, NO, 4 * P], BF, tag="h2t")
            eps128 = postsh.tile([P, 1], F, tag="eps128")
            nc.vector.memset(eps128, 1e-5)
            wp_sb = postsh.tile([P, 4, 2, C], F8, tag="wp")
            nc.sync.dma_start(out=wp_sb, in_=wp[:, :, :, :])
            xq_sb = postsh.tile([P, 4, C], F, tag="xqh")
            nc.sync.dma_start(
                out=xq_sb,
                in_=xq.rearrange("(rt p) c -> p rt c", p=P))

            w1s_cm = tc.tile_pool(name="w1stream", bufs=3, side="left")
            w1sp = w1s_cm.__enter__()
            pre_w1 = {}
            for hg in range(2):
                w1c = w1sp.tile([P, NO, 512], BF, tag="w1c")
                nc.sync.dma_start(
                    out=w1c,
                    in_=w1[:, hg * 512:(hg + 1) * 512].rearrange(
                        "(o p) j -> p o j", p=P))
                pre_w1[(hg, 0)] = w1c

            # ---- interleave 2: attn1 heads with Wp(b0) + LN2 + transposes ----
            ln01 = _ln2_chunks(nc, (0, 1), eps_sb, h2t_sb, eps128, smp)
            tail0 = _wp_chunks(nc, 0, st, xq_sb, eps_sb, wp_sb, mmp, smp) + ln01
            for i in range(5, H):
                A1.head(i)
                if i - 5 < len(tail0):
                    tail0[i - 5]()
            A1.flush()
            for j in range(H - 5, len(tail0)):
                tail0[j]()
            kv1_cm.__exit__(None, None, None)

            pyp_cm.__exit__(None, None, None)
            psp_cm.__exit__(None, None, None)

            pacc_cm = tc.tile_pool(name="pacc", bufs=1, space="PSUM",
                                   side="right")
            paccp = pacc_cm.__enter__()
            ptr_cm = tc.tile_pool(name="ptrp", bufs=2, space="PSUM",
                                  side="right")
            ptrp = ptr_cm.__enter__()

            mlpsb_cm = tc.tile_pool(name="mlpsb", bufs=1, side="left")
            mlpp = mlpsb_cm.__enter__()
            aT_sb = mlpp.tile([P, D4 // P, 4 * P], BF, tag="aT")

            def tail1():
                for fn in _wp_chunks(nc, 1, st, xq_sb, eps_sb, wp_sb, mmp,
                                     smp):
                    fn()
                for fn in _ln2_chunks(nc, (2, 3), eps_sb, h2t_sb, eps128,
                                      smp):
                    fn()

            _mlp1_half(nc, h2t_sb, w1, aT_sb, ptrp, w1sp, pre_w1, 0,
                       between=tail1, mmp=mmp)
            mm_cm.__exit__(None, None, None)

            _mlp(nc, tc, eps_sb, h2t_sb, w1, w2, out, paccp, ptrp,
                 w1sp, pre_w1, mlpp, aT_sb, None)

            mlpsb_cm.__exit__(None, None, None)
            ptr_cm.__exit__(None, None, None)
            pacc_cm.__exit__(None, None, None)
            w1s_cm.__exit__(None, None, None)
            postsh_cm.__exit__(None, None, None)

    nc.compile()
    return nc


def _kvq_tiles(nc, b, st, hT, wk_sb, wv_sb, mmp, xp, after_first_dma=None,
               k_eng=None, v_eng=None):
    """K^T and V projections for batch b over all token tiles (fp8 DR)."""
    v_sb = st["v"][b]
    v_heads = v_sb.rearrange("p r (h w) -> p r h w", w=65)
    for tt in range(NTT):
        ts_ = tt * TT
        ht = xp.tile([P, 4, 2, TT], F8, tag="ht")
        if tt == 0 and after_first_dma is not None:
            # split loads: o=0 slices land first so the first K chain starts early
            nc.sync.dma_start(out=ht[:, 0:1],
                              in_=hT[:, 0:1, :, b * T + ts_:b * T + ts_ + TT])
            after_first_dma()
            nc.sync.dma_start(out=ht[:, 1:4],
                              in_=hT[:, 1:4, :, b * T + ts_:b * T + ts_ + TT])
        else:
            nc.sync.dma_start(out=ht,
                              in_=hT[:, :, :, b * T + ts_:b * T + ts_ + TT])
        if tt == 0:
            # softmax-denominator column: WSV/16 (so yev = 16*y_true)
            nc.gpsimd.memset(v_heads[:, :, :, 64:65], WSV / 16.0)
        for q4 in range(4):
            _kvq_quarter(nc, b, st, ht, wk_sb, wv_sb, mmp, tt, q4,
                         k_eng, v_eng)


def _kvq_quarter(nc, b, st, ht, wk_sb, wv_sb, mmp, tt, q4, k_eng, v_eng):
    """A quarter token tile's worth of K (2 jt) + V (1 t4) projections."""
    k_sb, v_sb = st["k"][b], st["v"][b]
    v_heads = v_sb.rearrange("p r (h w) -> p r h w", w=65)
    ts_ = tt * TT
    for jj in range(2):
        jt = q4 * 2 + jj
        pk = mmp.tile([P, TT], F, tag="mm", name=f"pk{b}_{tt}_{jt}")
        for o in range(4):
            nc.tensor.matmul(pk, wk_sb[:, o, :, jt * P:(jt + 1) * P],
                             ht[:, o], start=(o == 0), stop=(o == 3),
                             perf_mode=DR)
        _copy(nc, k_eng[jt % len(k_eng)], k_sb[:, jt, ts_:ts_ + TT], pk)
    t4 = q4
    ridx = tt * (TT // P) + t4
    for nh in range(2):
        pv = mmp.tile([P, 512], F, tag="mm", name=f"pv{b}_{ridx}_{nh}")
        for o in range(4):
            nc.tensor.matmul(pv, ht[:, o, :, t4 * P:(t4 + 1) * P],
                             wv_sb[:, o, :, nh * 512:(nh + 1) * 512],
                             start=(o == 0), stop=(o == 3),
                             perf_mode=DR)
        pvv = pv.rearrange("p (h d) -> p h d", d=HD)
        _copy(nc, v_eng[nh % len(v_eng)],
              v_heads[:, ridx, nh * 8:(nh + 1) * 8, 0:HD], pvv)


def _q_proj(nc, b, st, hqT, wq_sb, mmp, xp):
    qT_sb = st["q"][b]
    hq = xp.tile([P, 4, 2, 2 * P], F8, tag="hq")
    nc.sync.dma_start(out=hq, in_=hqT[:, :, :, b * 2 * P:(b + 1) * 2 * P])
    for jt in range(NO):
        pq = mmp.tile([P, 2 * P], F, tag="mm", name=f"pq{b}_{jt}")
        for o in range(4):
            nc.tensor.matmul(pq, wq_sb[:, o, :, jt * P:(jt + 1) * P],
                             hq[:, o], start=(o == 0), stop=(o == 3),
                             perf_mode=DR)
        _copy(nc, "dve", qT_sb[:, jt, :], pq)


def _kvq_step_list(nc, b, st, hT, hqT, wk_sb, wv_sb, wq_sb, mmp, xp):
    """kvq for batch b as 16 callables (one per half token tile), for
    interleaving with attention heads. Evictions balanced for the
    concurrent-attention engine budget."""
    k_eng = ("dve",)
    v_eng = ("dve",)
    hts = {}
    steps = []
    for tt in range(NTT):
        for q4 in range(4):
            def mk(tt=tt, q4=q4):
                def fn():
                    if q4 == 0:
                        ts_ = tt * TT
                        ht = xp.tile([P, 4, 2, TT], F8, tag="ht")
                        nc.sync.dma_start(
                            out=ht,
                            in_=hT[:, :, :, b * T + ts_:b * T + ts_ + TT])
                        if tt == 0:
                            v_heads = st["v"][b].rearrange(
                                "p r (h w) -> p r h w", w=65)
                            nc.gpsimd.memset(v_heads[:, :, :, 64:65], WSV / 16.0)
                        hts[tt] = ht
                    _kvq_quarter(nc, b, st, hts[tt], wk_sb, wv_sb, mmp, tt,
                                 q4, k_eng, v_eng)
                    if tt == NTT - 1 and q4 == 3:
                        _q_proj(nc, b, st, hqT, wq_sb, mmp, xp)
                return fn
            steps.append(mk())
    return steps


class _AttnState:
    """Software-pipelined attention for one batch: issue scores/exp/mask for
    head h, then AV for head h-1."""

    def __init__(self, nc, b, st, mask_sb, smp, pyp, ptp, smtp, ystp, psp,
                 pool_kb2=(2, 3), pool_kb4=(1,)):
        self.nc = nc
        self.b = b
        self.st = st
        self.mask_sb = mask_sb
        self.smp, self.pyp, self.ptp = smp, pyp, ptp
        self.smtp, self.ystp, self.psp = smtp, ystp, psp
        self.pool_kb2, self.pool_kb4 = pool_kb2, pool_kb4
        self.pending = []

    def head(self, h):
        pts = self._scores(h)
        self.pending.append((h, pts))
        if len(self.pending) > 2:
            self._av(*self.pending.pop(0))

    def flush(self):
        while self.pending:
            self._av(*self.pending.pop(0))

    def _scores(self, h):
        nc, b = self.nc, self.b
        k_sb, qT_sb = self.st["k"][b], self.st["q"][b]
        mask_sb = self.mask_sb
        po = (h % 2) * 64
        jt = h // 2
        pts = []
        for kb2 in range(4):
            ps_ = self.psp.tile([P, 512], F, tag="ps", name=f"ps{b}_{h}_{kb2}")
            for half in range(2):
                kb = kb2 * 2 + half
                nc.tensor.matmul(ps_[:, half * 256:(half + 1) * 256],
                                 k_sb[po:po + 64, jt, kb * P:(kb + 1) * P],
                                 qT_sb[po:po + 64, jt, :], start=True, stop=True)
            pe = self.smtp.tile([P, 512], BF, tag="pe")
            nc.scalar.activation(out=pe, in_=ps_, func=AF.Exp,
                                 scale=1.0 / (WS * WS))
            pT = self.ptp.tile([P, 512], BF, tag="pT")
            eng = nc.gpsimd if kb2 in self.pool_kb2 else nc.vector
            eng.tensor_tensor(
                out=pT, in0=pe,
                in1=mask_sb[:, kb2 * 2:kb2 * 2 + 2, :].rearrange("p a q -> p (a q)"),
                op=OP.mult)
            pts.append(pT)
        for kb4 in range(2):
            ps_ = self.psp.tile([P, 512], F, tag="ps", name=f"psB{b}_{h}_{kb4}")
            for j in range(4):
                kb = 8 + kb4 * 4 + j
                nc.tensor.matmul(ps_[:, j * P:(j + 1) * P],
                                 k_sb[po:po + 64, jt, kb * P:(kb + 1) * P],
                                 qT_sb[po:po + 64, jt, P:2 * P],
                                 start=True, stop=True)
            pe = self.smtp.tile([P, 512], BF, tag="pe")
            nc.scalar.activation(out=pe, in_=ps_, func=AF.Exp,
                                 scale=1.0 / (WS * WS))
            pT = self.ptp.tile([P, 512], BF, tag="pT")
            eng = nc.gpsimd if kb4 in self.pool_kb4 else nc.vector
            eng.tensor_tensor(
                out=pT.rearrange("p (a q) -> p a q", a=4),
                in0=pe.rearrange("p (a q) -> p a q", a=4),
                in1=mask_sb[:, 8 + kb4 * 4:8 + (kb4 + 1) * 4, P:2 * P],
                op=OP.mult)
            pts.append(pT)
        return pts

    def _av(self, h, pts):
        nc, b = self.nc, self.b
        v_sb, yT_sb = self.st["v"][b], self.st["y"][b]
        py = self.pyp.tile([65, 2 * P], F, tag="py")
        for kb2 in range(4):
            for half in range(2):
                kb = kb2 * 2 + half
                nc.tensor.matmul(py, v_sb[:, kb, h * 65:h * 65 + 65],
                                 pts[kb2][:, half * 256:(half + 1) * 256],
                                 start=(kb == 0), stop=False)
        for kb4 in range(2):
            for j in range(4):
                kb = 8 + kb4 * 4 + j
                nc.tensor.matmul(py[:, P:2 * P],
                                 v_sb[:, kb, h * 65:h * 65 + 65],
                                 pts[4 + kb4][:, j * P:(j + 1) * P],
                                 start=False, stop=(kb == NBLK - 1))
        rec = self.smp.tile([1, 2 * P], F, tag="rec")
        nc.vector.reciprocal(out=rec, in_=py[64:65, :])
        recb = self.smp.tile([64, 2 * P], F, tag="recb")
        nc.gpsimd.partition_broadcast(recb, rec)
        yev = self.ystp.tile([64, 2 * P], F8, tag="yev")
        nc.vector.tensor_tensor(out=yev, in0=py[0:64, :], in1=recb,
                                op=OP.mult)
        nc.sync.dma_start(
            out=yT_sb[(h % 2) * 64:(h % 2) * 64 + 64, h // 4, (h // 2) % 2, :],
            in_=yev)


def _wp_chunks(nc, b, st, xq_sb, eps_sb, wp_sb, mmp, smp):
    """Wp projection (fp8 DR) + residual for batch b, as 4 chunk callables."""
    yT_sb = st["y"][b]
    chunks = []
    for th in range(2):
        for nh in range(2):
            def mk(th=th, nh=nh):
                def fn():
                    rt = 2 * b + th
                    pr = mmp.tile([P, 512], F, tag="mm",
                                  name=f"pr{b}_{th}_{nh}")
                    for o in range(4):
                        nc.tensor.matmul(
                            pr, yT_sb[:, o, :, th * P:(th + 1) * P],
                            wp_sb[:, o, :, nh * 512:(nh + 1) * 512],
                            start=(o == 0), stop=(o == 3), perf_mode=DR)
                    nc.vector.scalar_tensor_tensor(
                        out=eps_sb[:, rt, nh * 512:(nh + 1) * 512],
                        in0=pr, scalar=1.0 / (WS * 16.0),
                        in1=xq_sb[:, rt, nh * 512:(nh + 1) * 512],
                        op0=OP.mult, op1=OP.add)
                return fn
            chunks.append(mk())
    return chunks


def _ln2_chunks(nc, rts, eps_sb, h2t_sb, eps128, h2p):
    """LN2 stats+normalize and h2 DMA-transpose for the given rt row-blocks,
    as callables."""
    chunks = []
    for rt in rts:
        def mk_ln(rt=rt):
            def fn():
                stats = h2p.tile([P, 2, 6], F, tag="st2")
                nc.vector.bn_stats(out=stats[:, 0, :], in_=eps_sb[:, rt, 0:512])
                nc.vector.bn_stats(out=stats[:, 1, :],
                                   in_=eps_sb[:, rt, 512:1024])
                mv = h2p.tile([P, 2], F, tag="mv2")
                nc.vector.bn_aggr(out=mv, in_=stats)
                sd = h2p.tile([P, 1], F, tag="sd2")
                nc.scalar.activation(out=sd, in_=mv[:, 1:2], func=AF.Sqrt,
                                     bias=eps128)
                rstd2 = h2p.tile([P, 1], F, tag="rstd2")
                nc.vector.reciprocal(out=rstd2, in_=sd)
                h2 = h2p.tile([P, C], BF, tag=f"h2_{rt % 2}")
                nc.vector.tensor_scalar(out=h2, in0=eps_sb[:, rt, :],
                                        scalar1=mv[:, 0:1], scalar2=rstd2,
                                        op0=OP.subtract, op1=OP.mult)
                nc.sync.dma_start_transpose(
                    out=h2t_sb[:, :, rt * P:(rt + 1) * P], in_=h2)
            return fn
        chunks.append(mk_ln())
    return chunks


def _mlp1_half(nc, h2t_sb, w1, aT_sb, ptrp, wsp, pre_w1, half, between=None,
               mmp=None):
    """MLP1 for one 256-token half: aT[:, :, half] = gelu(W1^T @ h2T-half)."""
    cs = half * 2 * P
    for hg in range(D4 // 512):
        if (hg, half) in pre_w1:
            w1c = pre_w1[(hg, half)]
        else:
            w1c = wsp.tile([P, NO, 512], BF, tag="w1c")
            nc.sync.dma_start(
                out=w1c,
                in_=w1[:, hg * 512:(hg + 1) * 512].rearrange(
                    "(o p) j -> p o j", p=P))
            pre_w1[(hg, half)] = None
        for hi in range(4):
            ht = hg * 4 + hi
            if mmp is not None and hg < 2:
                pa = mmp.tile([P, TT], F, tag="mm",
                              name=f"pam{ht}_{half}")[:, 0:2 * P]
            else:
                pa = ptrp.tile([P, 2 * P], F, tag="pa", name=f"pa{ht}_{half}")
            for o in range(NO):
                nc.tensor.matmul(pa, w1c[:, o, hi * P:(hi + 1) * P],
                                 h2t_sb[:, o, cs:cs + 2 * P],
                                 start=(o == 0), stop=(o == NO - 1))
            nc.scalar.activation(out=aT_sb[:, ht, cs:cs + 2 * P], in_=pa,
                                 func=AF.Gelu)
        if between is not None and hg == 0:
            between()
            between = None


def _mlp(nc, tc, eps_sb, h2t_sb, w1, w2, out, paccp, ptrp, wsp, pre_w1,
         mlpp, aT_sb, between):
    """MLP over all 512 own rows + final residual + output DMA (bf16)."""
    if True:
        # ---- MLP1: aT = gelu(W1^T @ h2T), second half ----
        _mlp1_half(nc, h2t_sb, w1, aT_sb, ptrp, wsp, pre_w1, 1,
                   between=between)

        # ---- MLP2 + residual ----
        macc_cm = tc.tile_pool(name="macc", bufs=1, space="PSUM", side="left")
        maccp = macc_cm.__enter__()
        with tc.tile_pool(name="w2stream", bufs=2, side="left") as wsp2:
            out_sb = mlpp.tile([P, 4, C], F, tag="outsb")
            for nh in range(2):
                pms = [paccp.tile([P, 512], F, tag=f"acc{rt}",
                                  name=f"pm{nh}_{rt}")
                       for rt in range(2)]
                pms += [maccp.tile([P, 512], F, tag=f"acc{rt}",
                                   name=f"pm{nh}_{rt}")
                        for rt in range(2, 4)]
                for hg in range(D4 // 512):
                    w2c = wsp2.tile([P, 4, 512], BF, tag="w2c",
                                    name=f"w2c{nh}_{hg}")
                    nc.sync.dma_start(
                        out=w2c,
                        in_=w2[hg * 512:(hg + 1) * 512, nh * 512:(nh + 1) * 512]
                        .rearrange("(g p) j -> p g j", p=P))
                    for gi in range(4):
                        hc = hg * 4 + gi
                        for rt in range(4):
                            nc.tensor.matmul(
                                pms[rt], aT_sb[:, hc, rt * P:(rt + 1) * P],
                                w2c[:, gi, :],
                                start=(hc == 0), stop=(hc == D4 // P - 1))
                for rt in range(4):
                    nc.vector.tensor_tensor(
                        out=out_sb[:, rt, nh * 512:(nh + 1) * 512],
                        in0=pms[rt],
                        in1=eps_sb[:, rt, nh * 512:(nh + 1) * 512],
                        op=OP.add)
                    nc.sync.dma_start(
                        out=out.rearrange("(rt p) c -> p rt c", p=P)[
                            :, rt:rt + 1, nh * 512:(nh + 1) * 512],
                        in_=out_sb[:, rt:rt + 1, nh * 512:(nh + 1) * 512])
        macc_cm.__exit__(None, None, None)


def _dr_pack(m):
    """[C_in, N] -> [128, 4, 2, N], channel c -> (c//256, (c%256)//128, c%128)."""
    cin, n = m.shape
    assert cin == C
    return np.ascontiguousarray(m.reshape(4, 2, P, n).transpose(2, 0, 1, 3))


def _host_prep(inputs):
    """Host-side LN1, fp8 DR packing, per-core in_maps."""
    ii = {k: np.asarray(v, dtype=np.float32) for k, v in inputs.items()}
    x = ii["x"]
    for bias in ("bq", "bk", "bv", "bp", "b1", "b2", "ln1_b", "ln2_b"):
        assert np.allclose(ii[bias], 0.0), f"nonzero {bias} unsupported"

    e4 = ml_dtypes.float8_e4m3fn
    xflat = x.reshape(BT, C)
    mu = xflat.mean(axis=1, keepdims=True)
    var = ((xflat - mu) ** 2).mean(axis=1, keepdims=True)
    h = (xflat - mu) / np.sqrt(var + 1e-5)

    g1 = ii["ln1_g"][:, None]
    wq_f = (g1 * ii["Wq"] / np.sqrt(HD)).astype(np.float32)
    wk_f = (g1 * ii["Wk"]).astype(np.float32)
    wv_f = (g1 * ii["Wv"]).astype(np.float32)
    g2 = ii["ln2_g"][:, None]
    w1_f = (g2 * ii["W1"]).astype(np.float32)

    hT = np.ascontiguousarray(h.T)  # [C, BT]

    shared = {
        "hT": _dr_pack(hT).astype(e4),
        "wk": _dr_pack(wk_f * WS).astype(e4),
        "wq": _dr_pack(wq_f * WS).astype(e4),
        "wv": _dr_pack(wv_f * WSV).astype(e4),
        "wp": _dr_pack(ii["Wp"] * WS).astype(e4),
        "w1": np.ascontiguousarray(w1_f.astype(ml_dtypes.bfloat16)),
        "w2": np.ascontiguousarray(ii["W2"].astype(ml_dtypes.bfloat16)),
    }

    in_maps = []
    core_rows = []
    kk = np.arange(P)[:, None]
    jj = np.arange(2 * P)[None, :]
    for core in range(NCORES):
        qbA, qbB = core, NBLK - 1 - core
        rows = np.concatenate([
            b * T + qb * P + np.arange(P)
            for b in range(B) for qb in (qbA, qbB)])
        core_rows.append(rows)
        xq_i = np.ascontiguousarray(xflat[rows])
        hq_i = np.ascontiguousarray(h[rows].T)  # [C, 512]
        qpos = np.where(jj < P, qbA * P + jj, qbB * P + (jj - P))
        m = np.empty((NBLK, P, 2 * P), np.float32)
        for kb in range(NBLK):
            kpos = kb * P + kk
            m[kb] = np.where(kpos <= qpos, 1.0, 0.0)
        in_maps.append(dict(
            shared, xq=xq_i,
            hqT=_dr_pack(hq_i).astype(e4),
            mask=np.ascontiguousarray(m.astype(ml_dtypes.bfloat16))))
    return in_maps, core_rows


def kernel(**inputs):
    if "nc" not in _CACHE:
        _CACHE["nc"] = _build_program()
    nc = _CACHE["nc"]
    in_maps, core_rows = _host_prep(inputs)
    res = run_bass_kernel_spmd(nc, in_maps, core_ids=list(range(NCORES)))
    out = np.empty((BT, C), np.float32)
    for core in range(NCORES):
        out[core_rows[core]] = res.results[core]["out"]
    return out.reshape(B, T, C)


if __name__ == "__main__":
    print("module loads OK")
